# revision 1
# baseline (speedup 1.0000x reference)
"""CrossAttentionBlock Trainium2 kernel.

Math (reference):
    q = Wq@xq + bq        [RC=16, N]     (per-voxel 1x1x1 conv == channel matmul)
    k = Wk@xkv + bk       [16, N]
    v = Wv@xkv + bv       [C=128, N]
    S = (q^T k) / 4       [N, N]
    P = softmax_rows(S)
    out = v @ P^T         [C, N]
    y = x_q + gamma*out

Kernel strategy (8 NeuronCores, sequence-parallel over the N=13824 query
tokens; each core owns NQ=1728 queries against full K/V):
  * Host folds: 1/sqrt(RC) into Wq/bq; gamma into Wv; gamma*bv + x_q into the
    residual (softmax rows sum to 1 so the v-bias is a per-channel constant).
  * Scores are built TRANSPOSED (S^T tiles [128 keys x 432 queries]): k-tile
    stationary, q moving - no transposes anywhere.  Softmax needs no max
    subtraction (|S|<~3 by construction) and normalization is deferred:
    exp(S^T) feeds two accumulating matmuls - out_u = (gamma*v)^T-contracted
    output and a ones-row matmul giving row sums - and the divide happens once
    at the end via reciprocal + a 1->128 partition-broadcast matmul.
  * All three inner matmuls (S^T, out_u, rowsum) run fp8e4 + DoubleRow (2
    MACs/cell/cycle).  K/Q live in the DoubleRow layout [Ki=8, Ko=2, *]
    (virtual row r = p + 8o, staged via an SBUF->SBUF DMA partition remap);
    the out_u/rowsum moving operand pairs two consecutive key tiles.
  * exp is the throughput limit (191M elements through 1-elem/cycle/lane
    engines), so it is split ~53/47 between ScalarE (true exp, fp8 out) and
    VectorE (Schraudolph int8 bit-trick writing e4m3 bit patterns).  To
    amortize each engine's fixed per-op cost, S^T tiles live in a manual
    6-slot single-PSUM-bank arena and exp runs on 3 slots at a time with a
    single strided access pattern, writing a 12-slot SBUF fp8 ring that the
    matmuls consume in pairs.  Attention contributes O(1e-4) of the output
    magnitude, so ~6% fp8 quantization is invisible; the residual is fp32.
"""

import contextlib

import numpy as np
import ml_dtypes

import concourse.bass as bass
import concourse.mybir as mybir
from concourse import bacc
from concourse.tile import TileContext
from concourse.bass_utils import run_bass_kernel_spmd

F32 = mybir.dt.float32
BF16 = mybir.dt.bfloat16
FP8 = mybir.dt.float8e4
I8 = mybir.dt.int8
AF = mybir.ActivationFunctionType
DR = mybir.MatmulPerfMode.DoubleRow

C = 128           # channels
RC = 16           # reduced (q/k) channels
D = H = W = 24
N = D * H * W     # 13824 tokens
NCORES = 8
NQ = N // NCORES  # 1728 queries per core
CHUNK = 432       # query chunk ([128, CHUNK] fp32 fits one PSUM bank)
NCHUNKS = NQ // CHUNK   # 4
MT = N // 128     # 108 key tiles of 128
PAIRS = MT // 2   # 54 key-tile pairs per chunk
LAGP = 6          # out/rs matmuls trail exp by this many pairs (PE is in-order;
                  # the lag must cover exp latency with PE work or PE stalls)

LOG2E = 1.4426950408889634
EXP8_SCALE = 8.0 * LOG2E      # e4m3: 3 mantissa bits, bias 7
EXP8_BIAS = 56.0 - 0.3        # 7*8 + Schraudolph offset
# exp pair -> engine: Bresenham-interleaved so ScalarE/VectorE overlap
# (runs of the same engine would serialize the whole pipeline)
ACT_FRAC = 0.53


def _act_pattern(n):
    pat, acc = [], 0.0
    for _ in range(n):
        acc += ACT_FRAC
        if acc >= 1.0:
            acc -= 1.0
            pat.append(True)
        else:
            pat.append(False)
    return pat

_BUILD_CACHE: dict = {}


def build_nc(repeats: int = 1):
    """Build + compile the per-core Bass program (SPMD across 8 cores)."""
    key = repeats
    if key in _BUILD_CACHE:
        return _BUILD_CACHE[key]

    nc = bacc.Bacc("TRN2", target_bir_lowering=False, debug=False,
                   num_devices=NCORES)
    xq = nc.dram_tensor("xq", [C, NQ], F32, kind="ExternalInput").ap()
    xkv = nc.dram_tensor("xkv", [C, N], BF16, kind="ExternalInput").ap()
    wqT = nc.dram_tensor("wqT", [C, RC], BF16, kind="ExternalInput").ap()
    wkT = nc.dram_tensor("wkT", [C, RC], BF16, kind="ExternalInput").ap()
    wvT = nc.dram_tensor("wvT", [C, C], BF16, kind="ExternalInput").ap()
    bq = nc.dram_tensor("bq", [RC, 1], F32, kind="ExternalInput").ap()
    bk = nc.dram_tensor("bk", [RC, 1], F32, kind="ExternalInput").ap()
    y = nc.dram_tensor("y", [C, NQ], F32, kind="ExternalOutput").ap()

    with TileContext(nc) as tc, contextlib.ExitStack() as ctx:
        cpool = ctx.enter_context(tc.tile_pool(name="consts", bufs=1))
        ppool = ctx.enter_context(tc.tile_pool(name="psum", bufs=1, space="PSUM"))
        spool = ctx.enter_context(tc.tile_pool(name="work", bufs=1))

        # ---- resident inputs -------------------------------------------------
        xq_sb = cpool.tile([C, NQ], F32)
        nc.sync.dma_start(xq_sb[:], xq[:])
        xkv_sb = cpool.tile([C, N], BF16)
        nc.sync.dma_start(xkv_sb[:], xkv[:])
        wqT_sb = cpool.tile([C, RC], BF16)
        nc.sync.dma_start(wqT_sb[:], wqT[:])
        wkT_sb = cpool.tile([C, RC], BF16)
        nc.sync.dma_start(wkT_sb[:], wkT[:])
        wvT_sb = cpool.tile([C, C], BF16)
        nc.sync.dma_start(wvT_sb[:], wvT[:])
        bq_sb = cpool.tile([RC, 1], F32)
        nc.sync.dma_start(bq_sb[:], bq[:])
        bk_sb = cpool.tile([RC, 1], F32)
        nc.sync.dma_start(bk_sb[:], bk[:])

        # lhsT for DoubleRow row-sum matmul; padded so the Ko step is 16B
        # (ISA requires step%16==0 on the DoubleRow stationary AP)
        ones_db = cpool.tile([C, 32], FP8)
        nc.gpsimd.memset(ones_db[:], 1.0)
        ones_row = cpool.tile([1, C], BF16)   # lhsT for 1->128 broadcast matmul
        nc.gpsimd.memset(ones_row[:], 1.0)

        # ---- projections -----------------------------------------------------
        xq_bf = cpool.tile([C, NQ], BF16)
        nc.gpsimd.tensor_copy(xq_bf[:], xq_sb[:])

        # Prologue psum traffic rotates through the S^T pair-supertile slots
        # AND the (not-yet-live) outu/rs bank slots - 5 banks of pipelining
        # for the projection evacuations instead of 3.
        _pcnt = [0]

        def slot_ap(parts, width):
            i = _pcnt[0] % 5
            _pcnt[0] += 1
            if i < 3:
                t = ppool.tile([C, 1024], F32, tag="st", bufs=3, name="pslot")
            elif i == 3:
                t = ppool.tile([C, 512], F32, tag="outu", bufs=1, name="pslot_o")
            else:
                t = ppool.tile([C, 512], F32, tag="rs", bufs=1, name="pslot_r")
            return t[0:parts, 0:width]

        k_tmp = cpool.tile([RC, N], FP8)
        for i in range(N // 512):
            sl = bass.ts(i, 512)
            psk = slot_ap(RC, 512)
            nc.tensor.matmul(psk, wkT_sb[:], xkv_sb[:, sl], start=True, stop=True)
            if i % 2 == 0:
                nc.scalar.activation(k_tmp[:, sl], psk, AF.Identity, bias=bk_sb[:])
            else:
                nc.vector.tensor_scalar(out=k_tmp[:, sl], in0=psk,
                                        scalar1=bk_sb[:], scalar2=None,
                                        op0=mybir.AluOpType.add)

        q_tmp = cpool.tile([RC, NQ], FP8)
        for ch in range(NCHUNKS):
            sl = bass.ts(ch, CHUNK)
            psq = slot_ap(RC, CHUNK)
            nc.tensor.matmul(psq, wqT_sb[:], xq_bf[:, sl], start=True, stop=True)
            nc.scalar.activation(q_tmp[:, sl], psq, AF.Identity, bias=bq_sb[:])

        # DoubleRow layout [8, 2, *]: virtual row r = p + 8*o.  k_db DMAs are
        # split so early key tiles unlock before the whole projection lands.
        QN = N // 4
        k_db = cpool.tile([8, 2 * N], FP8)
        for qq in range(4):
            lo, hi = qq * QN, (qq + 1) * QN
            nc.sync.dma_start(k_db[:, lo:hi], k_tmp[0:8, lo:hi])
            nc.sync.dma_start(k_db[:, N + lo:N + hi], k_tmp[8:16, lo:hi])
        q_db = cpool.tile([8, 2 * NQ], FP8)
        nc.sync.dma_start(q_db[:, 0:NQ], q_tmp[0:8, :])
        nc.sync.dma_start(q_db[:, NQ:2 * NQ], q_tmp[8:16, :])
        q3 = q_db.rearrange("p (o x) -> p o x", o=2)
        k3 = k_db.rearrange("p (o x) -> p o x", o=2)

        # v^T tiles (tile t: [m_local(128), c] = gamma*v[c, 128t+m]), evacuated
        # from PSUM four tiles per op to amortize the fixed engine cost.
        vt_sb = cpool.tile([C, N], FP8)
        for qd in range(MT // 4):
            psv = slot_ap(C, 512)
            for j in range(4):
                t = 4 * qd + j
                nc.tensor.matmul(psv[:, bass.ts(j, 128)], xkv_sb[:, bass.ts(t, 128)],
                                 wvT_sb[:], start=True, stop=True)
            dst = vt_sb[:, bass.ts(qd, 512)]
            if qd % 2 == 0:
                nc.scalar.copy(dst, psv[:])
            else:
                nc.vector.tensor_copy(dst, psv[:])

        # ---- attention main loop --------------------------------------------
        # The per-chunk normalize+residual epilogue is deferred into the NEXT
        # chunk's pipeline (two stages) so its PE/ACT ops never head-of-line
        # block the steady-state stream.
        act_pat = _act_pattern(NCHUNKS * PAIRS * max(repeats, 1))
        pend = {}

        def epi_a():
            # free outu/rs as early as possible
            pend["outu_s"] = outu_s = spool.tile([C, CHUNK], F32, name="outu_s",
                                                 tag="outu_s", bufs=2)
            nc.scalar.copy(outu_s[:], pend.pop("outu")[:])
            recip = spool.tile([1, CHUNK], F32, tag="recip", bufs=2)
            nc.vector.reciprocal_approx_fast(out=recip[:], in_=pend.pop("rs")[:])
            pend["recip_bf"] = recip_bf = spool.tile([1, CHUNK], BF16,
                                                     name="recip_bf",
                                                     tag="recipb", bufs=2)
            nc.gpsimd.tensor_copy(recip_bf[:], recip[:])

        def epi_b():
            sl = pend.pop("sl")
            bcpt = ppool.tile([C, 1024], F32, tag="st", bufs=3, name="bcpt")
            bcp = bcpt[:, 0:CHUNK]
            nc.tensor.matmul(bcp, ones_row[:], pend.pop("recip_bf")[:],
                             start=True, stop=True)
            bcs = spool.tile([C, CHUNK], F32, tag="bcs", bufs=2)
            nc.scalar.copy(bcs[:], bcp)
            t1 = spool.tile([C, CHUNK], F32, tag="t1", bufs=2)
            nc.gpsimd.tensor_mul(t1[:], pend.pop("outu_s")[:], bcs[:])
            res = spool.tile([C, CHUNK], F32, tag="res", bufs=2)
            nc.gpsimd.tensor_add(res[:], t1[:], xq_sb[:, sl])
            nc.sync.dma_start(y[:, sl], res[:])

        for rep in range(repeats):
            for ch in range(NCHUNKS):
                sl = bass.ts(ch, CHUNK)
                outu = ppool.tile([C, CHUNK], F32, tag="outu")
                rs = ppool.tile([1, CHUNK], F32, tag="rs")
                gidx = (rep * NCHUNKS + ch) * PAIRS
                ex_tiles = {}
                for up in range(PAIRS + LAGP):
                    if up == 1 and "outu" in pend:
                        epi_a()
                    if up == 5 and "recip_bf" in pend:
                        epi_b()
                    if up < PAIRS:
                        s = up
                        stp = ppool.tile([C, 1024], F32, tag="st", bufs=3)
                        for j in range(2):
                            t = 2 * s + j
                            nc.tensor.matmul(stp[:, 512 * j:512 * j + CHUNK],
                                             k3[:, :, bass.ts(t, 128)],
                                             q3[:, :, sl],
                                             start=True, stop=True, perf_mode=DR)
                        st3 = stp.rearrange("p (b x) -> p b x", b=2)[:, :, 0:CHUNK]
                        ex = spool.tile([C, 2 * CHUNK], FP8, tag="ex", bufs=LAGP + 3)
                        ex3 = ex.rearrange("p (b x) -> p b x", b=2)
                        if act_pat[gidx + s]:
                            nc.scalar.activation(ex3, st3, AF.Exp)
                        else:
                            nc.vector.tensor_scalar(
                                out=ex3.bitcast(I8), in0=st3,
                                scalar1=EXP8_SCALE, scalar2=EXP8_BIAS,
                                op0=mybir.AluOpType.mult,
                                op1=mybir.AluOpType.add)
                        ex_tiles[s] = ex
                    if up >= LAGP:
                        s = up - LAGP
                        ex = ex_tiles.pop(s)
                        ex3 = ex.rearrange("p (b x) -> p b x", b=2)
                        vt3 = vt_sb[:, bass.ds(256 * s, 256)].rearrange(
                            "p (b c) -> p b c", b=2)
                        nc.tensor.matmul(outu[:], vt3, ex3, perf_mode=DR,
                                         start=(s == 0), stop=(s == PAIRS - 1))
                        o3 = ones_db.rearrange("p (b c) -> p b c", b=2)[:, :, 0:1]
                        nc.tensor.matmul(rs[:], o3, ex3, perf_mode=DR,
                                         start=(s == 0), stop=(s == PAIRS - 1))
                pend.update(outu=outu, rs=rs, sl=sl)
            if rep != repeats - 1:
                epi_a()
                epi_b()
                tc.strict_bb_all_engine_barrier()
        if "outu" in pend:
            epi_a()
        if "recip_bf" in pend:
            epi_b()

    nc.compile()
    _BUILD_CACHE[key] = nc
    return nc


def _prep_in_maps(x_q, x_kv, Wq, bq, Wk, bk, Wv, bv, gamma):
    bf16 = ml_dtypes.bfloat16
    f32 = np.float32
    x_q = np.asarray(x_q, f32).reshape(C, N)
    x_kv = np.asarray(x_kv, f32).reshape(C, N)
    Wq = np.asarray(Wq, f32)
    bq = np.asarray(bq, f32)
    Wk = np.asarray(Wk, f32)
    bk = np.asarray(bk, f32)
    Wv = np.asarray(Wv, f32)
    bv = np.asarray(bv, f32)
    gamma = float(np.asarray(gamma, f32).reshape(()))

    scale = 1.0 / np.sqrt(np.float32(RC))
    xkv_b = np.ascontiguousarray(x_kv).astype(bf16)
    wqT = np.ascontiguousarray(Wq.T * scale).astype(bf16)
    wkT = np.ascontiguousarray(Wk.T).astype(bf16)
    wvT = np.ascontiguousarray(Wv.T * gamma).astype(bf16)
    bq_s = np.ascontiguousarray((bq * scale).reshape(RC, 1))
    bk_s = np.ascontiguousarray(bk.reshape(RC, 1))
    resid_bias = (gamma * bv).astype(f32)  # softmax rows sum to 1

    in_maps = []
    for c in range(NCORES):
        xq_slice = np.ascontiguousarray(
            x_q[:, c * NQ:(c + 1) * NQ] + resid_bias[:, None], f32)
        in_maps.append({
            "xq": xq_slice, "xkv": xkv_b,
            "wqT": wqT, "wkT": wkT, "wvT": wvT,
            "bq": bq_s, "bk": bk_s,
        })
    return in_maps


def kernel(x_q, x_kv, Wq, bq, Wk, bk, Wv, bv, gamma):
    nc = build_nc(repeats=1)
    in_maps = _prep_in_maps(x_q, x_kv, Wq, bq, Wk, bk, Wv, bv, gamma)
    res = run_bass_kernel_spmd(nc, in_maps, list(range(NCORES)))
    out = np.concatenate([res.results[c]["y"] for c in range(NCORES)], axis=1)
    return out.reshape(1, C, D, H, W).astype(np.float32)



# revision 12
# speedup vs baseline: 1.1073x; 1.1073x over previous
"""CrossAttentionBlock Trainium2 kernel (v2).

Math (reference):
    q = Wq@xq + bq        [RC=16, N]     (per-voxel 1x1x1 conv == channel matmul)
    k = Wk@xkv + bk       [16, N]
    v = Wv@xkv + bv       [C=128, N]
    S = (q^T k) / 4       [N, N]
    P = softmax_rows(S)
    out = v @ P^T         [C, N]
    y = x_q + gamma*out

Kernel strategy (8 NeuronCores, sequence-parallel over the N=13824 query
tokens; each core owns NQ=1728 queries against full K/V):
  * The hard throughput floor is PSUM->SBUF evacuation bandwidth: only the
    Activation and DVE engines can read PSUM, and every exp'd score element
    must cross once (exp is fused into the evacuation op, so exp itself is
    free).  Everything else is arranged to keep that path minimal:
      - v is never materialized: out = (gamma*Wv) @ (xkv @ P^T) reassociated,
        so the big [C,N] v evacuation disappears; Z = xkv @ exp(S^T)
        accumulates in PSUM via the same per-pair matmuls and is evacuated
        once per chunk ([C,432] instead of [C,N]).
      - the k projection packs 3 column-groups of 16 output rows into one
        [128,512] PSUM tile (base partitions 0/32/64), so its evacuation
        runs with full 128-lane utilization; a few SBUF->SBUF DMAs remap to
        the DoubleRow layout afterwards.
  * Scores are built TRANSPOSED (S^T tiles [128 keys x 432 queries]), fp8 +
    DoubleRow everywhere (2 MACs/cell/cycle): k is host-scaled by 16 into
    fp8 weights, so exp applies a 1/16 input scale (ScalarE scale arg /
    folded into the Schraudolph constant on DVE).  No max subtraction
    (|S|<~2 by construction); normalization deferred: Z and a ones-row
    matmul (ones=0.25 folds the Z evac scale) accumulate per chunk, then
    reciprocal + 1->128 broadcast matmul + multiply + residual add.
  * exp alternates ScalarE (true exp, fp8 out) / VectorE (Schraudolph int8
    bit-trick) ~53/47 Bresenham-interleaved.  Inputs land fp8/bf16 (xkv in
    both [c,m] and [m,c] layouts), chunked DMAs so projections overlap the
    loads.  Attention contributes O(1e-4) of the output, so fp8 noise is
    invisible; the residual is bf16 (0.2% of tolerance).
"""

import contextlib

import numpy as np
import ml_dtypes

import concourse.bass as bass
import concourse.mybir as mybir
from concourse import bacc
from concourse.tile import TileContext
from concourse.bass_utils import run_bass_kernel_spmd

F32 = mybir.dt.float32
BF16 = mybir.dt.bfloat16
FP8 = mybir.dt.float8e4
I8 = mybir.dt.int8
AF = mybir.ActivationFunctionType
DR = mybir.MatmulPerfMode.DoubleRow
ALU = mybir.AluOpType

C = 128           # channels
RC = 16           # reduced (q/k) channels
D = H = W = 24
N = D * H * W     # 13824 tokens
NCORES = 8
NQ = N // NCORES  # 1728 queries per core
CHUNK = 432       # query chunk ([128, CHUNK] f32 fits half a PSUM slot)
NCHUNKS = NQ // CHUNK   # 4
MT = N // 128     # 108 key tiles of 128
PAIRS = MT // 2   # 54 key-tile pairs per chunk
LAGP = 6          # Z/rs matmuls trail exp by this many pairs

KTW = 512         # k-projection column width per matmul (one PSUM bank)
KGROUPS = 3       # k output-row groups per PSUM tile (base partitions 0/32/64)
KTILES = N // (KTW * KGROUPS)   # 9

LOG2E = 1.4426950408889634
EXP8_SCALE = 8.0 * LOG2E / 16.0   # e4m3 bit trick, folding the S'=16*S scale
EXP8_BIAS = 56.0 - 0.3            # 7*8 + Schraudolph offset
ACT_FRAC = 0.531                  # ScalarE share of exp ops (1025/(905+1025))


def _act_pattern(n):
    pat, acc = [], 0.0
    for _ in range(n):
        acc += ACT_FRAC
        if acc >= 1.0:
            acc -= 1.0
            pat.append(True)
        else:
            pat.append(False)
    return pat

_BUILD_CACHE: dict = {}


def build_nc(repeats: int = 1):
    """Build + compile the per-core Bass program (SPMD across 8 cores)."""
    key = repeats
    if key in _BUILD_CACHE:
        return _BUILD_CACHE[key]

    nc = bacc.Bacc("TRN2", target_bir_lowering=False, debug=False,
                   num_devices=NCORES)
    wbf = nc.dram_tensor("wbf", [C, 2 * RC + C], BF16, kind="ExternalInput").ap()
    bias2 = nc.dram_tensor("bias2", [C, 2], F32, kind="ExternalInput").ap()
    xkv_f8 = nc.dram_tensor("xkv_f8", [C, N], FP8, kind="ExternalInput").ap()
    xq_bf = nc.dram_tensor("xq_bf", [C, NQ], BF16, kind="ExternalInput").ap()
    xkvT = nc.dram_tensor("xkvT", [C, N], FP8, kind="ExternalInput").ap()
    y = nc.dram_tensor("y", [C, NQ], BF16, kind="ExternalOutput").ap()

    with TileContext(nc) as tc, contextlib.ExitStack() as ctx:
        cpool = ctx.enter_context(tc.tile_pool(name="consts", bufs=1))
        ppool = ctx.enter_context(tc.tile_pool(name="psum", bufs=1, space="PSUM"))
        spool = ctx.enter_context(tc.tile_pool(name="work", bufs=1))

        # ---- resident inputs (issue order == HWDGE order: critical first) --
        wbf_sb = cpool.tile([C, 2 * RC + C], BF16)
        nc.sync.dma_start(wbf_sb[:], wbf[:])
        bias_sb = cpool.tile([C, 2], F32)
        nc.sync.dma_start(bias_sb[:], bias2[:])
        xkv_sb = cpool.tile([C, N], FP8)
        XKC = N // 4
        nc.sync.dma_start(xkv_sb[:, 0:XKC], xkv_f8[:, 0:XKC])
        xq_sb = cpool.tile([C, NQ], BF16)
        nc.sync.dma_start(xq_sb[:], xq_bf[:])
        for qq in range(1, 4):
            sl = bass.ts(qq, XKC)
            nc.sync.dma_start(xkv_sb[:, sl], xkv_f8[:, sl])
        xkvT_sb = cpool.tile([C, N], FP8)
        for qq in range(4):
            sl = bass.ts(qq, N // 4)
            nc.sync.dma_start(xkvT_sb[:, sl], xkvT[:, sl])

        wqT = wbf_sb[:, 0:RC]
        wkT = wbf_sb[:, RC:2 * RC]
        wvT = wbf_sb[:, 2 * RC:2 * RC + C]
        bk16 = bias_sb[:, 0:1]
        bq4 = bias_sb[:, 1:2]

        ones_db = cpool.tile([C, 32], FP8)
        nc.gpsimd.memset(ones_db[:], 0.0625)   # folds the Z-evac 1/16 scale
        ones_row = cpool.tile([1, C], BF16)  # lhsT for 1->128 broadcast matmul
        nc.gpsimd.memset(ones_row[:], 1.0)

        # ---- projections ---------------------------------------------------
        # k': [128, KTILES*KTW] fp8, partition 32g+r holds 16*k[r] for column
        # group g; evacuations run full-width, then SBUF->SBUF DMAs remap to
        # the DoubleRow layout.
        k_sb = cpool.tile([C, KTILES * KTW], FP8)
        for t in range(KTILES):
            psk = ppool.tile([C, 2 * KTW], F32, tag="st", bufs=3, name="pk")
            for g in range(KGROUPS):
                lo = (KGROUPS * t + g) * KTW
                nc.tensor.matmul(psk[32 * g:32 * g + RC, 0:KTW],
                                 wkT, xkv_sb[:, lo:lo + KTW],
                                 start=True, stop=True)
            dst = k_sb[:, bass.ts(t, KTW)]
            if t % 2 == 0:
                nc.scalar.activation(dst, psk[:, 0:KTW], AF.Identity, bias=bk16)
            else:
                nc.vector.tensor_scalar(out=dst, in0=psk[:, 0:KTW],
                                        scalar1=bk16, scalar2=None, op0=ALU.add)

        k_db = cpool.tile([8, 2 * N], FP8)
        kv = k_sb.rearrange("p (t m) -> p t m", t=KTILES)
        kdv = k_db.rearrange("p (o t g m) -> p o t g m", o=2, t=KTILES, g=KGROUPS)
        for g in range(KGROUPS):
            for o in range(2):
                nc.sync.dma_start(kdv[:, o, :, g, :], kv[32 * g + 8 * o:32 * g + 8 * o + 8, :, :])

        # q': [128, CHUNK] tiles; groups of 16 rows at base partitions
        # 0/32/64, 3 groups in tile 0 and the 4th group alone in tile 1.
        q_sb0 = cpool.tile([C, CHUNK], FP8)
        q_sb1 = cpool.tile([C, CHUNK], FP8)
        psq0 = ppool.tile([C, 2 * KTW], F32, tag="st", bufs=3, name="pq0")
        for g in range(3):
            nc.tensor.matmul(psq0[32 * g:32 * g + RC, 0:CHUNK], wqT,
                             xq_sb[:, bass.ts(g, CHUNK)], start=True, stop=True)
        nc.scalar.activation(q_sb0[:], psq0[:, 0:CHUNK], AF.Identity, bias=bq4)
        psq1 = ppool.tile([C, 2 * KTW], F32, tag="st", bufs=3, name="pq1")
        nc.tensor.matmul(psq1[0:RC, 0:CHUNK], wqT, xq_sb[:, bass.ts(3, CHUNK)],
                         start=True, stop=True)
        nc.vector.tensor_scalar(out=q_sb1[:], in0=psq1[:, 0:CHUNK],
                                scalar1=bq4, scalar2=None, op0=ALU.add)

        q_db = cpool.tile([8, 2 * NQ], FP8)
        qdv = q_db.rearrange("p (o g m) -> p o g m", o=2, g=NCHUNKS)
        for g in range(NCHUNKS):
            src = q_sb0 if g < 3 else q_sb1
            base = 32 * g if g < 3 else 0
            for o in range(2):
                nc.sync.dma_start(qdv[:, o, g, :], src[base + 8 * o:base + 8 * o + 8, :])
        q3 = q_db.rearrange("p (o x) -> p o x", o=2)
        k3 = k_db.rearrange("p (o x) -> p o x", o=2)

        # ---- attention main loop ------------------------------------------
        # Per-chunk epilogue is deferred into the NEXT chunk's pipeline so it
        # never head-of-line blocks the steady-state stream.
        act_pat = _act_pattern(NCHUNKS * PAIRS * max(repeats, 1))
        pend = {}

        def epi_a():
            # evacuate Z (frees its bank for the next chunk) + reciprocal
            pend["z8"] = z8 = spool.tile([C, CHUNK], FP8, tag="z8", bufs=2,
                                         name="z8")
            nc.scalar.activation(z8[:], pend.pop("z")[:], AF.Copy,
                                 scale=1.0 / 16.0)
            recip = spool.tile([1, CHUNK], F32, tag="recip", bufs=2)
            nc.vector.reciprocal_approx_fast(out=recip[:], in_=pend.pop("rs")[:])
            pend["recip_bf"] = recip_bf = spool.tile([1, CHUNK], BF16,
                                                     tag="recipb", bufs=2,
                                                     name="recip_bf")
            nc.gpsimd.tensor_copy(recip_bf[:], recip[:])

        def epi_m():
            # borrow one st slot: outu (cols 0:CHUNK) + bcp (cols 512:512+CHUNK)
            pend["pe_t"] = pet = ppool.tile([C, 1024], F32, tag="st", bufs=3,
                                            name="pe_t")
            nc.tensor.matmul(pet[:, 0:CHUNK], wvT, pend.pop("z8")[:],
                             start=True, stop=True)
            nc.tensor.matmul(pet[:, 512:512 + CHUNK], ones_row[:],
                             pend.pop("recip_bf")[:], start=True, stop=True)
            pend["outu_s"] = outu_s = spool.tile([C, CHUNK], F32, tag="outu_s",
                                                 bufs=2, name="outu_s")
            nc.scalar.copy(outu_s[:], pet[:, 0:CHUNK])

        def epi_b():
            sl = pend.pop("sl")
            pet = pend.pop("pe_t")
            t1 = spool.tile([C, CHUNK], BF16, tag="t1", bufs=2)
            nc.vector.tensor_tensor(out=t1[:], in0=pet[:, 512:512 + CHUNK],
                                    in1=pend.pop("outu_s")[:], op=ALU.mult)
            res = spool.tile([C, CHUNK], BF16, tag="res", bufs=2)
            nc.vector.tensor_tensor(out=res[:], in0=t1[:], in1=xq_sb[:, sl],
                                    op=ALU.add)
            nc.sync.dma_start(y[:, sl], res[:])

        for rep in range(repeats):
            for ch in range(NCHUNKS):
                sl = bass.ts(ch, CHUNK)
                z = ppool.tile([C, CHUNK], F32, tag="z")
                rs = ppool.tile([1, CHUNK], F32, tag="rs")
                gidx = (rep * NCHUNKS + ch) * PAIRS
                ex_tiles = {}
                for up in range(PAIRS + LAGP):
                    if up == 1 and "z" in pend:
                        epi_a()
                    if up == 3 and "z8" in pend:
                        epi_m()
                    if up == 5 and "pe_t" in pend:
                        epi_b()
                    if up < PAIRS:
                        s = up
                        stp = ppool.tile([C, 1024], F32, tag="st", bufs=3)
                        for j in range(2):
                            t = 2 * s + j
                            nc.tensor.matmul(stp[:, 512 * j:512 * j + CHUNK],
                                             k3[:, :, bass.ts(t, 128)],
                                             q3[:, :, sl],
                                             start=True, stop=True, perf_mode=DR)
                        st3 = stp.rearrange("p (b x) -> p b x", b=2)[:, :, 0:CHUNK]
                        ex = spool.tile([C, 2 * CHUNK], FP8, tag="ex", bufs=LAGP + 3)
                        ex3 = ex.rearrange("p (b x) -> p b x", b=2)
                        if act_pat[gidx + s]:
                            nc.scalar.activation(ex3, st3, AF.Exp, scale=1.0 / 16.0)
                        else:
                            nc.vector.tensor_scalar(
                                out=ex3.bitcast(I8), in0=st3,
                                scalar1=EXP8_SCALE, scalar2=EXP8_BIAS,
                                op0=ALU.mult, op1=ALU.add)
                        ex_tiles[s] = ex
                    if up >= LAGP:
                        s = up - LAGP
                        ex = ex_tiles.pop(s)
                        ex3 = ex.rearrange("p (b x) -> p b x", b=2)
                        xt3 = xkvT_sb[:, bass.ds(256 * s, 256)].rearrange(
                            "p (b c) -> p b c", b=2)
                        nc.tensor.matmul(z[:], xt3, ex3, perf_mode=DR,
                                         start=(s == 0), stop=(s == PAIRS - 1))
                        o3 = ones_db.rearrange("p (b c) -> p b c", b=2)[:, :, 0:1]
                        nc.tensor.matmul(rs[:], o3, ex3, perf_mode=DR,
                                         start=(s == 0), stop=(s == PAIRS - 1))
                pend.update(z=z, rs=rs, sl=sl)
            if rep != repeats - 1:
                epi_a()
                epi_m()
                epi_b()
                tc.strict_bb_all_engine_barrier()
        if "z" in pend:
            epi_a()
        if "z8" in pend:
            epi_m()
        if "pe_t" in pend:
            epi_b()

    nc.compile()
    _BUILD_CACHE[key] = nc
    return nc


def _prep_in_maps(x_q, x_kv, Wq, bq, Wk, bk, Wv, bv, gamma):
    bf16 = ml_dtypes.bfloat16
    f8 = ml_dtypes.float8_e4m3
    f32 = np.float32
    x_q = np.asarray(x_q, f32).reshape(C, N)
    x_kv = np.asarray(x_kv, f32).reshape(C, N)
    Wq = np.asarray(Wq, f32)
    bq = np.asarray(bq, f32)
    Wk = np.asarray(Wk, f32)
    bk = np.asarray(bk, f32)
    Wv = np.asarray(Wv, f32)
    bv = np.asarray(bv, f32)
    gamma = float(np.asarray(gamma, f32).reshape(()))

    xkv_f8 = np.ascontiguousarray(x_kv).astype(f8)
    # xkv transposed [m, c] tiled by 128 keys (Z matmul stationary)
    xkvT = np.ascontiguousarray(
        x_kv.reshape(C, MT, 128).transpose(2, 1, 0).reshape(128, MT * C)).astype(f8)
    # bf16 weights blob: Wq^T/4 | 16*Wk^T | gamma*Wv^T
    wbf = np.zeros((C, 2 * RC + C), f32)
    wbf[:, 0:RC] = Wq.T * 0.25
    wbf[:, RC:2 * RC] = Wk.T * 16.0
    wbf[:, 2 * RC:2 * RC + C] = (gamma * Wv).T
    wbf = np.ascontiguousarray(wbf).astype(bf16)
    # biases: col0 = 16*bk tiled at rows 32g+r ; col1 = bq/4 tiled
    bias2 = np.zeros((C, 2), f32)
    for g in range(4):
        bias2[32 * g:32 * g + RC, 0] = 16.0 * bk
        bias2[32 * g:32 * g + RC, 1] = 0.25 * bq
    resid_bias = (gamma * bv).astype(f32)  # softmax rows sum to 1

    in_maps = []
    for c in range(NCORES):
        xq_slice = np.ascontiguousarray(
            x_q[:, c * NQ:(c + 1) * NQ] + resid_bias[:, None]).astype(bf16)
        in_maps.append({
            "wbf": wbf, "bias2": bias2,
            "xkv_f8": xkv_f8, "xq_bf": xq_slice, "xkvT": xkvT,
        })
    return in_maps


def kernel(x_q, x_kv, Wq, bq, Wk, bk, Wv, bv, gamma):
    nc = build_nc(repeats=1)
    in_maps = _prep_in_maps(x_q, x_kv, Wq, bq, Wk, bk, Wv, bv, gamma)
    res = run_bass_kernel_spmd(nc, in_maps, list(range(NCORES)))
    out = np.concatenate([res.results[c]["y"].astype(np.float32)
                          for c in range(NCORES)], axis=1)
    return out.reshape(1, C, D, H, W).astype(np.float32)


# revision 20
# speedup vs baseline: 1.1247x; 1.0157x over previous
"""CrossAttentionBlock Trainium2 kernel (v2).

Math (reference):
    q = Wq@xq + bq        [RC=16, N]     (per-voxel 1x1x1 conv == channel matmul)
    k = Wk@xkv + bk       [16, N]
    v = Wv@xkv + bv       [C=128, N]
    S = (q^T k) / 4       [N, N]
    P = softmax_rows(S)
    out = v @ P^T         [C, N]
    y = x_q + gamma*out

Kernel strategy (8 NeuronCores, sequence-parallel over the N=13824 query
tokens; each core owns NQ=1728 queries against full K/V):
  * The hard throughput floor is PSUM->SBUF evacuation bandwidth: only the
    Activation and DVE engines can read PSUM, and every exp'd score element
    must cross once (exp is fused into the evacuation op, so exp itself is
    free).  Everything else is arranged to keep that path minimal:
      - v is never materialized: out = (gamma*Wv) @ (xkv @ P^T) reassociated,
        so the big [C,N] v evacuation disappears; Z = xkv @ exp(S^T)
        accumulates in PSUM via the same per-pair matmuls and is evacuated
        once per chunk ([C,432] instead of [C,N]).
      - the k projection packs 3 column-groups of 16 output rows into one
        [128,512] PSUM tile (base partitions 0/32/64), so its evacuation
        runs with full 128-lane utilization; a few SBUF->SBUF DMAs remap to
        the DoubleRow layout afterwards.
  * Scores are built TRANSPOSED (S^T tiles [128 keys x 432 queries]), fp8 +
    DoubleRow everywhere (2 MACs/cell/cycle): k is host-scaled by 16 into
    fp8 weights, so exp applies a 1/16 input scale (ScalarE scale arg /
    folded into the Schraudolph constant on DVE).  No max subtraction
    (|S|<~2 by construction); normalization deferred: Z and a ones-row
    matmul (ones=0.25 folds the Z evac scale) accumulate per chunk, then
    reciprocal + 1->128 broadcast matmul + multiply + residual add.
  * exp alternates ScalarE (true exp, fp8 out) / VectorE (Schraudolph int8
    bit-trick) ~53/47 Bresenham-interleaved.  Inputs land fp8/bf16 (xkv in
    both [c,m] and [m,c] layouts), chunked DMAs so projections overlap the
    loads.  Attention contributes O(1e-4) of the output, so fp8 noise is
    invisible; the residual is bf16 (0.2% of tolerance).
"""

import contextlib

import numpy as np
import ml_dtypes

import concourse.bass as bass
import concourse.mybir as mybir
from concourse import bacc
from concourse.tile import TileContext
from concourse.bass_utils import run_bass_kernel_spmd

F32 = mybir.dt.float32
BF16 = mybir.dt.bfloat16
FP8 = mybir.dt.float8e4
I8 = mybir.dt.int8
AF = mybir.ActivationFunctionType
DR = mybir.MatmulPerfMode.DoubleRow
ALU = mybir.AluOpType

C = 128           # channels
RC = 16           # reduced (q/k) channels
D = H = W = 24
N = D * H * W     # 13824 tokens
NCORES = 8
NQ = N // NCORES  # 1728 queries per core
CHUNK = 432       # query chunk ([128, CHUNK] f32 fits half a PSUM slot)
NCHUNKS = NQ // CHUNK   # 4
MT = N // 128     # 108 key tiles of 128
PAIRS = MT // 2   # 54 key-tile pairs per chunk
LAGP = 6          # Z/rs matmuls trail exp by this many pairs

KTW = 512         # k-projection column width per matmul (one PSUM bank)
KGROUPS = 3       # k output-row groups per PSUM tile (base partitions 0/32/64)
KTILES = N // (KTW * KGROUPS)   # 9

LOG2E = 1.4426950408889634
EXP8_SCALE = 8.0 * LOG2E / 16.0   # e4m3 bit trick, folding the S'=16*S scale
EXP8_BIAS = 56.0 - 0.3            # 7*8 + Schraudolph offset
ACT_FRAC = 0.531                  # ScalarE share of exp ops (1025/(905+1025))


def _act_pattern(n):
    pat, acc = [], 0.0
    for _ in range(n):
        acc += ACT_FRAC
        if acc >= 1.0:
            acc -= 1.0
            pat.append(True)
        else:
            pat.append(False)
    return pat

_BUILD_CACHE: dict = {}


def build_nc(repeats: int = 1):
    """Build + compile the per-core Bass program (SPMD across 8 cores)."""
    key = repeats
    if key in _BUILD_CACHE:
        return _BUILD_CACHE[key]

    nc = bacc.Bacc("TRN2", target_bir_lowering=False, debug=False,
                   num_devices=NCORES)
    wbf = nc.dram_tensor("wbf", [C, 2 * RC + C], BF16, kind="ExternalInput").ap()
    bias2 = nc.dram_tensor("bias2", [C, 3], F32, kind="ExternalInput").ap()
    xkv_f8 = nc.dram_tensor("xkv_f8", [C, N], FP8, kind="ExternalInput").ap()
    xq_bf = nc.dram_tensor("xq_bf", [C, NQ], BF16, kind="ExternalInput").ap()
    xkvT = nc.dram_tensor("xkvT", [C, N], FP8, kind="ExternalInput").ap()
    y = nc.dram_tensor("y", [C, NQ], BF16, kind="ExternalOutput").ap()

    with TileContext(nc) as tc, contextlib.ExitStack() as ctx:
        cpool = ctx.enter_context(tc.tile_pool(name="consts", bufs=1))
        ppool = ctx.enter_context(tc.tile_pool(name="psum", bufs=1, space="PSUM"))
        spool = ctx.enter_context(tc.tile_pool(name="work", bufs=1))

        # ---- resident inputs (issue order == HWDGE order: critical first) --
        wbf_sb = cpool.tile([C, 2 * RC + C], BF16)
        nc.sync.dma_start(wbf_sb[:], wbf[:])
        bias_sb = cpool.tile([C, 3], F32)
        nc.sync.dma_start(bias_sb[:], bias2[:])
        xkv_sb = cpool.tile([C, N], FP8)
        XKC = N // 4
        nc.sync.dma_start(xkv_sb[:, 0:XKC], xkv_f8[:, 0:XKC])
        xq_sb = cpool.tile([C, NQ], BF16)
        nc.sync.dma_start(xq_sb[:], xq_bf[:])
        for qq in range(1, 4):
            sl = bass.ts(qq, XKC)
            nc.sync.dma_start(xkv_sb[:, sl], xkv_f8[:, sl])
        xkvT_sb = cpool.tile([C, N], FP8)
        for qq in range(4):
            sl = bass.ts(qq, N // 4)
            nc.sync.dma_start(xkvT_sb[:, sl], xkvT[:, sl])

        wqT = wbf_sb[:, 0:RC]
        wkT = wbf_sb[:, RC:2 * RC]
        wvT = wbf_sb[:, 2 * RC:2 * RC + C]
        bk16 = bias_sb[:, 0:1]
        bq_lo = bias_sb[0:8, 1:2]   # bq[p]/4 on partition p
        bq_hi = bias_sb[0:8, 2:3]   # bq[8+p]/4 on partition p

        ones_db = cpool.tile([C, 32], FP8)
        nc.gpsimd.memset(ones_db[:], 0.0625)   # folds the Z-evac 1/16 scale
        ones_row = cpool.tile([1, C], BF16)  # lhsT for 1->128 broadcast matmul
        nc.gpsimd.memset(ones_row[:], 1.0)
        warm_mv = cpool.tile([1, 512], BF16)
        nc.gpsimd.memset(warm_mv[:], 0.0)

        # PE p-state warmup: keep PE continuously busy from t~0 so the
        # projection matmuls run at full clock (ramp needs 3us of busy).
        warm_ps = ppool.tile([C, 512], F32, tag="rs", bufs=1, name="warm_ps")
        for _ in range(6):
            nc.tensor.matmul(warm_ps[0:1, :], ones_row[:, 0:1], warm_mv[:],
                             start=True, stop=True)

        # ---- projections ---------------------------------------------------
        # k': [128, KTILES*KTW] fp8, partition 32g+r holds 16*k[r] for column
        # group g; evacuations run full-width, then SBUF->SBUF DMAs remap to
        # the DoubleRow layout.
        k_sb = cpool.tile([C, KTILES * KTW], FP8)
        for t in range(KTILES):
            psk = ppool.tile([C, 2 * KTW], F32, tag="st", bufs=3, name="pk")
            for g in range(KGROUPS):
                lo = (KGROUPS * t + g) * KTW
                nc.tensor.matmul(psk[32 * g:32 * g + RC, 0:KTW],
                                 wkT, xkv_sb[:, lo:lo + KTW],
                                 start=True, stop=True)
            dst = k_sb[:, bass.ts(t, KTW)]
            if t % 2 == 0:
                nc.scalar.activation(dst, psk[:, 0:KTW], AF.Identity, bias=bk16)
            else:
                nc.vector.tensor_scalar(out=dst, in0=psk[:, 0:KTW],
                                        scalar1=bk16, scalar2=None, op0=ALU.add)

        # q': two 8-row matmuls per chunk-group write the DoubleRow halves
        # side by side in PSUM; one 8-lane evac lands straight in q_db layout
        # (no partition remap, no DMA).
        q_db = cpool.tile([8, 2 * NQ], FP8)
        qdv = q_db.rearrange("p (o g m) -> p o g m", o=2, g=NCHUNKS)
        for g in range(NCHUNKS):
            psq = ppool.tile([C, 2 * KTW], F32, tag="st", bufs=3, name="psq")
            for o in range(2):
                nc.tensor.matmul(psq[0:8, 512 * o:512 * o + CHUNK],
                                 wqT[:, 8 * o:8 * o + 8],
                                 xq_sb[:, bass.ts(g, CHUNK)],
                                 start=True, stop=True)
            for o, b in ((0, bq_lo), (1, bq_hi)):
                src = psq[0:8, 512 * o:512 * o + CHUNK]
                if (2 * g + o) % 2 == 0:
                    nc.scalar.activation(qdv[:, o, g, :], src, AF.Identity,
                                         bias=b)
                else:
                    nc.vector.tensor_scalar(out=qdv[:, o, g, :], in0=src,
                                            scalar1=b, scalar2=None,
                                            op0=ALU.add)

        # k remap DMAs: group 0 first (unblocks the first S^T pairs)
        k_db = cpool.tile([8, 2 * N], FP8)
        kv = k_sb.rearrange("p (t m) -> p t m", t=KTILES)
        kdv = k_db.rearrange("p (o t g m) -> p o t g m", o=2, t=KTILES, g=KGROUPS)
        for g in range(KGROUPS):
            for o in range(2):
                nc.sync.dma_start(kdv[:, o, :, g, :], kv[32 * g + 8 * o:32 * g + 8 * o + 8, :, :])

        q3 = q_db.rearrange("p (o x) -> p o x", o=2)
        k3 = k_db.rearrange("p (o x) -> p o x", o=2)

        # ---- attention main loop ------------------------------------------
        # Per-chunk epilogue is deferred into the NEXT chunk's pipeline so it
        # never head-of-line blocks the steady-state stream.
        act_pat = _act_pattern(NCHUNKS * PAIRS * max(repeats, 1))
        pend = {}

        def epi_a():
            # evacuate Z (frees its bank for the next chunk) + reciprocal
            pend["z8"] = z8 = spool.tile([C, CHUNK], FP8, tag="z8", bufs=2,
                                         name="z8")
            nc.scalar.activation(z8[:], pend.pop("z")[:], AF.Copy,
                                 scale=1.0 / 16.0)
            recip = spool.tile([1, CHUNK], F32, tag="recip", bufs=2)
            nc.vector.reciprocal_approx_fast(out=recip[:], in_=pend.pop("rs")[:])
            pend["recip_bf"] = recip_bf = spool.tile([1, CHUNK], BF16,
                                                     tag="recipb", bufs=2,
                                                     name="recip_bf")
            nc.gpsimd.tensor_copy(recip_bf[:], recip[:])

        def epi_m():
            # borrow one st slot: outu (cols 0:CHUNK) + bcp (cols 512:512+CHUNK)
            pend["pe_t"] = pet = ppool.tile([C, 1024], F32, tag="st", bufs=3,
                                            name="pe_t")
            nc.tensor.matmul(pet[:, 0:CHUNK], wvT, pend.pop("z8")[:],
                             start=True, stop=True)
            nc.tensor.matmul(pet[:, 512:512 + CHUNK], ones_row[:],
                             pend.pop("recip_bf")[:], start=True, stop=True)
            pend["outu_s"] = outu_s = spool.tile([C, CHUNK], F32, tag="outu_s",
                                                 bufs=2, name="outu_s")
            nc.scalar.copy(outu_s[:], pet[:, 0:CHUNK])

        def epi_b():
            sl = pend.pop("sl")
            pet = pend.pop("pe_t")
            t1 = spool.tile([C, CHUNK], BF16, tag="t1", bufs=2)
            nc.vector.tensor_tensor(out=t1[:], in0=pet[:, 512:512 + CHUNK],
                                    in1=pend.pop("outu_s")[:], op=ALU.mult)
            res = spool.tile([C, CHUNK], BF16, tag="res", bufs=2)
            nc.vector.tensor_tensor(out=res[:], in0=t1[:], in1=xq_sb[:, sl],
                                    op=ALU.add)
            nc.sync.dma_start(y[:, sl], res[:])

        for rep in range(repeats):
            for ch in range(NCHUNKS):
                sl = bass.ts(ch, CHUNK)
                z = ppool.tile([C, CHUNK], F32, tag="z")
                rs = ppool.tile([1, CHUNK], F32, tag="rs")
                gidx = (rep * NCHUNKS + ch) * PAIRS
                ex_tiles = {}
                for up in range(PAIRS + LAGP):
                    if up == 1 and "z" in pend:
                        epi_a()
                    if up == 3 and "z8" in pend:
                        epi_m()
                    if up == 5 and "pe_t" in pend:
                        epi_b()
                    if up < PAIRS:
                        s = up
                        stp = ppool.tile([C, 1024], F32, tag="st", bufs=3)
                        for j in range(2):
                            t = 2 * s + j
                            nc.tensor.matmul(stp[:, 512 * j:512 * j + CHUNK],
                                             k3[:, :, bass.ts(t, 128)],
                                             q3[:, :, sl],
                                             start=True, stop=True, perf_mode=DR)
                        st3 = stp.rearrange("p (b x) -> p b x", b=2)[:, :, 0:CHUNK]
                        ex = spool.tile([C, 2 * CHUNK], FP8, tag="ex", bufs=LAGP + 3)
                        ex3 = ex.rearrange("p (b x) -> p b x", b=2)
                        if act_pat[gidx + s]:
                            nc.scalar.activation(ex3, st3, AF.Exp, scale=1.0 / 16.0)
                        else:
                            nc.vector.tensor_scalar(
                                out=ex3.bitcast(I8), in0=st3,
                                scalar1=EXP8_SCALE, scalar2=EXP8_BIAS,
                                op0=ALU.mult, op1=ALU.add)
                        ex_tiles[s] = ex
                    if up >= LAGP:
                        s = up - LAGP
                        ex = ex_tiles.pop(s)
                        ex3 = ex.rearrange("p (b x) -> p b x", b=2)
                        xt3 = xkvT_sb[:, bass.ds(256 * s, 256)].rearrange(
                            "p (b c) -> p b c", b=2)
                        nc.tensor.matmul(z[:], xt3, ex3, perf_mode=DR,
                                         start=(s == 0), stop=(s == PAIRS - 1))
                        o3 = ones_db.rearrange("p (b c) -> p b c", b=2)[:, :, 0:1]
                        nc.tensor.matmul(rs[:], o3, ex3, perf_mode=DR,
                                         start=(s == 0), stop=(s == PAIRS - 1))
                pend.update(z=z, rs=rs, sl=sl)
            if rep != repeats - 1:
                epi_a()
                epi_m()
                epi_b()
                tc.strict_bb_all_engine_barrier()
        if "z" in pend:
            epi_a()
        if "z8" in pend:
            epi_m()
        if "pe_t" in pend:
            epi_b()

    nc.compile()
    _BUILD_CACHE[key] = nc
    return nc


def _prep_in_maps(x_q, x_kv, Wq, bq, Wk, bk, Wv, bv, gamma):
    bf16 = ml_dtypes.bfloat16
    f8 = ml_dtypes.float8_e4m3
    f32 = np.float32
    x_q = np.asarray(x_q, f32).reshape(C, N)
    x_kv = np.asarray(x_kv, f32).reshape(C, N)
    Wq = np.asarray(Wq, f32)
    bq = np.asarray(bq, f32)
    Wk = np.asarray(Wk, f32)
    bk = np.asarray(bk, f32)
    Wv = np.asarray(Wv, f32)
    bv = np.asarray(bv, f32)
    gamma = float(np.asarray(gamma, f32).reshape(()))

    xkv_f8 = np.ascontiguousarray(x_kv).astype(f8)
    # xkv transposed [m, c] tiled by 128 keys (Z matmul stationary)
    xkvT = np.ascontiguousarray(
        x_kv.reshape(C, MT, 128).transpose(2, 1, 0).reshape(128, MT * C)).astype(f8)
    # bf16 weights blob: Wq^T/4 | 16*Wk^T | gamma*Wv^T
    wbf = np.zeros((C, 2 * RC + C), f32)
    wbf[:, 0:RC] = Wq.T * 0.25
    wbf[:, RC:2 * RC] = Wk.T * 16.0
    wbf[:, 2 * RC:2 * RC + C] = (gamma * Wv).T
    wbf = np.ascontiguousarray(wbf).astype(bf16)
    # biases: col0 = 16*bk tiled at rows 32g+r ; col1/col2 = bq/4 halves
    bias2 = np.zeros((C, 3), f32)
    for g in range(4):
        bias2[32 * g:32 * g + RC, 0] = 16.0 * bk
    bias2[0:8, 1] = 0.25 * bq[0:8]
    bias2[0:8, 2] = 0.25 * bq[8:16]
    resid_bias = (gamma * bv).astype(f32)  # softmax rows sum to 1

    in_maps = []
    for c in range(NCORES):
        xq_slice = np.ascontiguousarray(
            x_q[:, c * NQ:(c + 1) * NQ] + resid_bias[:, None]).astype(bf16)
        in_maps.append({
            "wbf": wbf, "bias2": bias2,
            "xkv_f8": xkv_f8, "xq_bf": xq_slice, "xkvT": xkvT,
        })
    return in_maps


def kernel(x_q, x_kv, Wq, bq, Wk, bk, Wv, bv, gamma):
    nc = build_nc(repeats=1)
    in_maps = _prep_in_maps(x_q, x_kv, Wq, bq, Wk, bk, Wv, bv, gamma)
    res = run_bass_kernel_spmd(nc, in_maps, list(range(NCORES)))
    out = np.concatenate([res.results[c]["y"].astype(np.float32)
                          for c in range(NCORES)], axis=1)
    return out.reshape(1, C, D, H, W).astype(np.float32)


# revision 23
# speedup vs baseline: 1.1422x; 1.0156x over previous
"""CrossAttentionBlock Trainium2 kernel (v2).

Math (reference):
    q = Wq@xq + bq        [RC=16, N]     (per-voxel 1x1x1 conv == channel matmul)
    k = Wk@xkv + bk       [16, N]
    v = Wv@xkv + bv       [C=128, N]
    S = (q^T k) / 4       [N, N]
    P = softmax_rows(S)
    out = v @ P^T         [C, N]
    y = x_q + gamma*out

Kernel strategy (8 NeuronCores, sequence-parallel over the N=13824 query
tokens; each core owns NQ=1728 queries against full K/V):
  * The hard throughput floor is PSUM->SBUF evacuation bandwidth: only the
    Activation and DVE engines can read PSUM, and every exp'd score element
    must cross once (exp is fused into the evacuation op, so exp itself is
    free).  Everything else is arranged to keep that path minimal:
      - v is never materialized: out = (gamma*Wv) @ (xkv @ P^T) reassociated,
        so the big [C,N] v evacuation disappears; Z = xkv @ exp(S^T)
        accumulates in PSUM via the same per-pair matmuls and is evacuated
        once per chunk ([C,432] instead of [C,N]).
      - the k projection packs 3 column-groups of 16 output rows into one
        [128,512] PSUM tile (base partitions 0/32/64), so its evacuation
        runs with full 128-lane utilization; a few SBUF->SBUF DMAs remap to
        the DoubleRow layout afterwards.
  * Scores are built TRANSPOSED (S^T tiles [128 keys x 432 queries]), fp8 +
    DoubleRow everywhere (2 MACs/cell/cycle): k is host-scaled by 16 into
    fp8 weights, so exp applies a 1/16 input scale (ScalarE scale arg /
    folded into the Schraudolph constant on DVE).  No max subtraction
    (|S|<~2 by construction); normalization deferred: Z and a ones-row
    matmul (ones=0.25 folds the Z evac scale) accumulate per chunk, then
    reciprocal + 1->128 broadcast matmul + multiply + residual add.
  * exp alternates ScalarE (true exp, fp8 out) / VectorE (Schraudolph int8
    bit-trick) ~53/47 Bresenham-interleaved.  Inputs land fp8/bf16 (xkv in
    both [c,m] and [m,c] layouts), chunked DMAs so projections overlap the
    loads.  Attention contributes O(1e-4) of the output, so fp8 noise is
    invisible; the residual is bf16 (0.2% of tolerance).
"""

import contextlib

import numpy as np
import ml_dtypes

import concourse.bass as bass
import concourse.mybir as mybir
from concourse import bacc
from concourse.tile import TileContext
from concourse.bass_utils import run_bass_kernel_spmd

F32 = mybir.dt.float32
BF16 = mybir.dt.bfloat16
FP8 = mybir.dt.float8e4
I8 = mybir.dt.int8
AF = mybir.ActivationFunctionType
DR = mybir.MatmulPerfMode.DoubleRow
ALU = mybir.AluOpType

C = 128           # channels
RC = 16           # reduced (q/k) channels
D = H = W = 24
N = D * H * W     # 13824 tokens
NCORES = 8
NQ = N // NCORES  # 1728 queries per core
CHUNK = 432       # query chunk ([128, CHUNK] f32 fits half a PSUM slot)
NCHUNKS = NQ // CHUNK   # 4
MT = N // 128     # 108 key tiles of 128
PAIRS = MT // 2   # 54 key-tile pairs per chunk
LAGP = 6          # Z/rs matmuls trail exp by this many pairs

KTW = 512         # k-projection column width per matmul (one PSUM bank)
KGROUPS = 3       # k output-row groups per PSUM tile (base partitions 0/32/64)
KTILES = N // (KTW * KGROUPS)   # 9

LOG2E = 1.4426950408889634
EXP8_SCALE = 8.0 * LOG2E / 16.0   # e4m3 bit trick, folding the S'=16*S scale
EXP8_BIAS = 56.0 - 0.3            # 7*8 + Schraudolph offset
ACT_FRAC = 0.531                  # ScalarE share of exp ops (1025/(905+1025))


def _act_pattern(n):
    pat, acc = [], 0.0
    for _ in range(n):
        acc += ACT_FRAC
        if acc >= 1.0:
            acc -= 1.0
            pat.append(True)
        else:
            pat.append(False)
    return pat

_BUILD_CACHE: dict = {}


def build_nc(repeats: int = 1):
    """Build + compile the per-core Bass program (SPMD across 8 cores)."""
    key = repeats
    if key in _BUILD_CACHE:
        return _BUILD_CACHE[key]

    nc = bacc.Bacc("TRN2", target_bir_lowering=False, debug=False,
                   num_devices=NCORES)
    wbf = nc.dram_tensor("wbf", [C, 2 * RC + C], BF16, kind="ExternalInput").ap()
    bias2 = nc.dram_tensor("bias2", [C, 3], F32, kind="ExternalInput").ap()
    xkv_f8 = nc.dram_tensor("xkv_f8", [C, N], FP8, kind="ExternalInput").ap()
    xq_bf = nc.dram_tensor("xq_bf", [C, NQ], BF16, kind="ExternalInput").ap()
    xkvT = nc.dram_tensor("xkvT", [C, N], FP8, kind="ExternalInput").ap()
    y = nc.dram_tensor("y", [C, NQ], BF16, kind="ExternalOutput").ap()

    with TileContext(nc) as tc, contextlib.ExitStack() as ctx:
        cpool = ctx.enter_context(tc.tile_pool(name="consts", bufs=1))
        ppool = ctx.enter_context(tc.tile_pool(name="psum", bufs=1, space="PSUM"))
        spool = ctx.enter_context(tc.tile_pool(name="work", bufs=1))

        # ---- resident inputs (issue order == HWDGE order: critical first) --
        wbf_sb = cpool.tile([C, 2 * RC + C], BF16)
        nc.sync.dma_start(wbf_sb[:], wbf[:])
        bias_sb = cpool.tile([C, 3], F32)
        nc.sync.dma_start(bias_sb[:], bias2[:])
        xkv_sb = cpool.tile([C, N], FP8)
        XKC = N // 4
        nc.sync.dma_start(xkv_sb[:, 0:XKC], xkv_f8[:, 0:XKC])
        xq_sb = cpool.tile([C, NQ], BF16)
        nc.sync.dma_start(xq_sb[:], xq_bf[:])
        for qq in range(1, 4):
            sl = bass.ts(qq, XKC)
            nc.sync.dma_start(xkv_sb[:, sl], xkv_f8[:, sl])
        xkvT_sb = cpool.tile([C, N], FP8)
        for qq in range(4):
            sl = bass.ts(qq, N // 4)
            nc.sync.dma_start(xkvT_sb[:, sl], xkvT[:, sl])

        wqT = wbf_sb[:, 0:RC]
        wkT = wbf_sb[:, RC:2 * RC]
        wvT = wbf_sb[:, 2 * RC:2 * RC + C]
        bk16 = bias_sb[:, 0:1]
        bq_lo = bias_sb[0:8, 1:2]   # bq[p]/4 on partition p
        bq_hi = bias_sb[0:8, 2:3]   # bq[8+p]/4 on partition p

        ones_db = cpool.tile([C, 32], FP8)
        nc.gpsimd.memset(ones_db[:], 0.0625)   # folds the Z-evac 1/16 scale
        ones_row = cpool.tile([1, C], BF16)  # lhsT for 1->128 broadcast matmul
        nc.gpsimd.memset(ones_row[:], 1.0)
        warm_mv = cpool.tile([1, 512], BF16)
        nc.gpsimd.memset(warm_mv[:], 0.0)

        # PE p-state warmup: keep PE continuously busy from t~0 so the
        # projection matmuls run at full clock (ramp needs 3us of busy).
        warm_ps = ppool.tile([C, 512], F32, tag="rs", bufs=1, name="warm_ps")
        for _ in range(6):
            nc.tensor.matmul(warm_ps[0:1, :], ones_row[:, 0:1], warm_mv[:],
                             start=True, stop=True)

        # ---- projections ---------------------------------------------------
        # k': [128, KTILES*KTW] fp8, partition 32g+r holds 16*k[r] for column
        # group g; evacuations run full-width, then SBUF->SBUF DMAs remap to
        # the DoubleRow layout.  q': two 8-row matmuls per chunk-group write
        # the DoubleRow halves side by side in PSUM; 8-lane evacs land
        # straight in q_db layout (no remap DMA).  q-groups are interleaved
        # between k-tiles so chunk 0's queries are ready before the k remaps.
        k_sb = cpool.tile([C, KTILES * KTW], FP8)
        q_db = cpool.tile([8, 2 * NQ], FP8)
        qdv = q_db.rearrange("p (o g m) -> p o g m", o=2, g=NCHUNKS)

        def q_proj(g):
            psq = ppool.tile([C, 2 * KTW], F32, tag="st", bufs=3, name="psq")
            for o in range(2):
                nc.tensor.matmul(psq[0:8, 512 * o:512 * o + CHUNK],
                                 wqT[:, 8 * o:8 * o + 8],
                                 xq_sb[:, bass.ts(g, CHUNK)],
                                 start=True, stop=True)
            for o, b in ((0, bq_lo), (1, bq_hi)):
                src = psq[0:8, 512 * o:512 * o + CHUNK]
                if (2 * g + o) % 2 == 0:
                    nc.scalar.activation(qdv[:, o, g, :], src, AF.Identity,
                                         bias=b)
                else:
                    nc.vector.tensor_scalar(out=qdv[:, o, g, :], in0=src,
                                            scalar1=b, scalar2=None,
                                            op0=ALU.add)

        for t in range(KTILES):
            psk = ppool.tile([C, 2 * KTW], F32, tag="st", bufs=3, name="pk")
            for g in range(KGROUPS):
                lo = (KGROUPS * t + g) * KTW
                nc.tensor.matmul(psk[32 * g:32 * g + RC, 0:KTW],
                                 wkT, xkv_sb[:, lo:lo + KTW],
                                 start=True, stop=True)
            dst = k_sb[:, bass.ts(t, KTW)]
            if t % 2 == 0:
                nc.scalar.activation(dst, psk[:, 0:KTW], AF.Identity, bias=bk16)
            else:
                nc.vector.tensor_scalar(out=dst, in0=psk[:, 0:KTW],
                                        scalar1=bk16, scalar2=None, op0=ALU.add)
            if t in (2, 4, 6, 8):
                q_proj(t // 2 - 1)

        # k remap DMAs, split by t-half so the early half fires as soon as
        # the first four evacuations land; o=0 rides HWDGE (SP), o=1 rides
        # SWDGE (Pool) so the two halves transfer in parallel paths.
        k_db = cpool.tile([8, 2 * N], FP8)
        kv = k_sb.rearrange("p (t m) -> p t m", t=KTILES)
        kdv = k_db.rearrange("p (o t g m) -> p o t g m", o=2, t=KTILES, g=KGROUPS)
        for tsl in (slice(0, 4), slice(4, KTILES)):
            for g in range(KGROUPS):
                src0 = kv[32 * g:32 * g + 8, tsl, :]
                src1 = kv[32 * g + 8:32 * g + 16, tsl, :]
                nc.sync.dma_start(kdv[:, 0, tsl, g, :], src0)
                nc.gpsimd.dma_start(kdv[:, 1, tsl, g, :], src1)

        q3 = q_db.rearrange("p (o x) -> p o x", o=2)
        k3 = k_db.rearrange("p (o x) -> p o x", o=2)

        # ---- attention main loop ------------------------------------------
        # Per-chunk epilogue is deferred into the NEXT chunk's pipeline so it
        # never head-of-line blocks the steady-state stream.
        act_pat = _act_pattern(NCHUNKS * PAIRS * max(repeats, 1))
        pend = {}

        def epi_a():
            # evacuate Z (frees its bank for the next chunk) + reciprocal
            pend["z8"] = z8 = spool.tile([C, CHUNK], FP8, tag="z8", bufs=2,
                                         name="z8")
            nc.scalar.activation(z8[:], pend.pop("z")[:], AF.Copy,
                                 scale=1.0 / 16.0)
            recip = spool.tile([1, CHUNK], F32, tag="recip", bufs=2)
            nc.vector.reciprocal_approx_fast(out=recip[:], in_=pend.pop("rs")[:])
            pend["recip_bf"] = recip_bf = spool.tile([1, CHUNK], BF16,
                                                     tag="recipb", bufs=2,
                                                     name="recip_bf")
            nc.gpsimd.tensor_copy(recip_bf[:], recip[:])

        def epi_m():
            # borrow one st slot: outu (cols 0:CHUNK) + bcp (cols 512:512+CHUNK)
            pend["pe_t"] = pet = ppool.tile([C, 1024], F32, tag="st", bufs=3,
                                            name="pe_t")
            nc.tensor.matmul(pet[:, 0:CHUNK], wvT, pend.pop("z8")[:],
                             start=True, stop=True)
            nc.tensor.matmul(pet[:, 512:512 + CHUNK], ones_row[:],
                             pend.pop("recip_bf")[:], start=True, stop=True)
            pend["outu_s"] = outu_s = spool.tile([C, CHUNK], F32, tag="outu_s",
                                                 bufs=2, name="outu_s")
            nc.scalar.copy(outu_s[:], pet[:, 0:CHUNK])

        def epi_b():
            sl = pend.pop("sl")
            pet = pend.pop("pe_t")
            t1 = spool.tile([C, CHUNK], BF16, tag="t1", bufs=2)
            nc.vector.tensor_tensor(out=t1[:], in0=pet[:, 512:512 + CHUNK],
                                    in1=pend.pop("outu_s")[:], op=ALU.mult)
            res = spool.tile([C, CHUNK], BF16, tag="res", bufs=2)
            nc.gpsimd.tensor_add(res[:], t1[:], xq_sb[:, sl])
            nc.sync.dma_start(y[:, sl], res[:])

        def epi_final(ch):
            # tail-latency version: two column halves pipelined across engines
            z, rs = pend.pop("z"), pend.pop("rs")
            HW_ = CHUNK // 2
            for h in range(2):
                lo = h * HW_
                z8h = spool.tile([C, HW_], FP8, tag="z8f", bufs=2, name="z8f")
                nc.scalar.activation(z8h[:], z[:, lo:lo + HW_], AF.Copy,
                                     scale=1.0 / 16.0)
                rch = spool.tile([1, HW_], F32, tag="recf", bufs=2, name="rcf")
                nc.vector.reciprocal_approx_fast(out=rch[:],
                                                 in_=rs[:, lo:lo + HW_])
                rbh = spool.tile([1, HW_], BF16, tag="rbf", bufs=2, name="rbf")
                nc.gpsimd.tensor_copy(rbh[:], rch[:])
                pet = ppool.tile([C, 1024], F32, tag="st", bufs=3, name="pe_f")
                nc.tensor.matmul(pet[:, 0:HW_], wvT, z8h[:],
                                 start=True, stop=True)
                nc.tensor.matmul(pet[:, 512:512 + HW_], ones_row[:], rbh[:],
                                 start=True, stop=True)
                osh = spool.tile([C, HW_], F32, tag="osf", bufs=2, name="osf")
                nc.scalar.copy(osh[:], pet[:, 0:HW_])
                t1h = spool.tile([C, HW_], BF16, tag="t1f", bufs=2, name="t1f")
                nc.vector.tensor_tensor(out=t1h[:], in0=pet[:, 512:512 + HW_],
                                        in1=osh[:], op=ALU.mult)
                rsh = spool.tile([C, HW_], BF16, tag="resf", bufs=2, name="rsf")
                cl = bass.ds(ch * CHUNK + lo, HW_)
                nc.gpsimd.tensor_add(rsh[:], t1h[:], xq_sb[:, cl])
                nc.sync.dma_start(y[:, cl], rsh[:])

        for rep in range(repeats):
            for ch in range(NCHUNKS):
                sl = bass.ts(ch, CHUNK)
                z = ppool.tile([C, CHUNK], F32, tag="z")
                rs = ppool.tile([1, CHUNK], F32, tag="rs")
                gidx = (rep * NCHUNKS + ch) * PAIRS
                ex_tiles = {}
                for up in range(PAIRS + LAGP):
                    if up == 1 and "z" in pend:
                        epi_a()
                    if up == 3 and "z8" in pend:
                        epi_m()
                    if up == 5 and "pe_t" in pend:
                        epi_b()
                    if up < PAIRS:
                        s = up
                        stp = ppool.tile([C, 1024], F32, tag="st", bufs=3)
                        for j in range(2):
                            t = 2 * s + j
                            nc.tensor.matmul(stp[:, 512 * j:512 * j + CHUNK],
                                             k3[:, :, bass.ts(t, 128)],
                                             q3[:, :, sl],
                                             start=True, stop=True, perf_mode=DR)
                        st3 = stp.rearrange("p (b x) -> p b x", b=2)[:, :, 0:CHUNK]
                        ex = spool.tile([C, 2 * CHUNK], FP8, tag="ex", bufs=LAGP + 3)
                        ex3 = ex.rearrange("p (b x) -> p b x", b=2)
                        if act_pat[gidx + s]:
                            nc.scalar.activation(ex3, st3, AF.Exp, scale=1.0 / 16.0)
                        else:
                            nc.vector.tensor_scalar(
                                out=ex3.bitcast(I8), in0=st3,
                                scalar1=EXP8_SCALE, scalar2=EXP8_BIAS,
                                op0=ALU.mult, op1=ALU.add)
                        ex_tiles[s] = ex
                    if up >= LAGP:
                        s = up - LAGP
                        ex = ex_tiles.pop(s)
                        ex3 = ex.rearrange("p (b x) -> p b x", b=2)
                        xt3 = xkvT_sb[:, bass.ds(256 * s, 256)].rearrange(
                            "p (b c) -> p b c", b=2)
                        nc.tensor.matmul(z[:], xt3, ex3, perf_mode=DR,
                                         start=(s == 0), stop=(s == PAIRS - 1))
                        o3 = ones_db.rearrange("p (b c) -> p b c", b=2)[:, :, 0:1]
                        nc.tensor.matmul(rs[:], o3, ex3, perf_mode=DR,
                                         start=(s == 0), stop=(s == PAIRS - 1))
                pend.update(z=z, rs=rs, sl=sl)
            if rep != repeats - 1:
                epi_a()
                epi_m()
                epi_b()
                tc.strict_bb_all_engine_barrier()
        if "z" in pend:
            pend.pop("sl")
            epi_final(NCHUNKS - 1)

    nc.compile()
    _BUILD_CACHE[key] = nc
    return nc


def _prep_in_maps(x_q, x_kv, Wq, bq, Wk, bk, Wv, bv, gamma):
    bf16 = ml_dtypes.bfloat16
    f8 = ml_dtypes.float8_e4m3
    f32 = np.float32
    x_q = np.asarray(x_q, f32).reshape(C, N)
    x_kv = np.asarray(x_kv, f32).reshape(C, N)
    Wq = np.asarray(Wq, f32)
    bq = np.asarray(bq, f32)
    Wk = np.asarray(Wk, f32)
    bk = np.asarray(bk, f32)
    Wv = np.asarray(Wv, f32)
    bv = np.asarray(bv, f32)
    gamma = float(np.asarray(gamma, f32).reshape(()))

    xkv_f8 = np.ascontiguousarray(x_kv).astype(f8)
    # xkv transposed [m, c] tiled by 128 keys (Z matmul stationary)
    xkvT = np.ascontiguousarray(
        x_kv.reshape(C, MT, 128).transpose(2, 1, 0).reshape(128, MT * C)).astype(f8)
    # bf16 weights blob: Wq^T/4 | 16*Wk^T | gamma*Wv^T
    wbf = np.zeros((C, 2 * RC + C), f32)
    wbf[:, 0:RC] = Wq.T * 0.25
    wbf[:, RC:2 * RC] = Wk.T * 16.0
    wbf[:, 2 * RC:2 * RC + C] = (gamma * Wv).T
    wbf = np.ascontiguousarray(wbf).astype(bf16)
    # biases: col0 = 16*bk tiled at rows 32g+r ; col1/col2 = bq/4 halves
    bias2 = np.zeros((C, 3), f32)
    for g in range(4):
        bias2[32 * g:32 * g + RC, 0] = 16.0 * bk
    bias2[0:8, 1] = 0.25 * bq[0:8]
    bias2[0:8, 2] = 0.25 * bq[8:16]
    resid_bias = (gamma * bv).astype(f32)  # softmax rows sum to 1

    in_maps = []
    for c in range(NCORES):
        xq_slice = np.ascontiguousarray(
            x_q[:, c * NQ:(c + 1) * NQ] + resid_bias[:, None]).astype(bf16)
        in_maps.append({
            "wbf": wbf, "bias2": bias2,
            "xkv_f8": xkv_f8, "xq_bf": xq_slice, "xkvT": xkvT,
        })
    return in_maps


def kernel(x_q, x_kv, Wq, bq, Wk, bk, Wv, bv, gamma):
    nc = build_nc(repeats=1)
    in_maps = _prep_in_maps(x_q, x_kv, Wq, bq, Wk, bk, Wv, bv, gamma)
    res = run_bass_kernel_spmd(nc, in_maps, list(range(NCORES)))
    out = np.concatenate([res.results[c]["y"].astype(np.float32)
                          for c in range(NCORES)], axis=1)
    return out.reshape(1, C, D, H, W).astype(np.float32)


# revision 27
# speedup vs baseline: 1.1501x; 1.0070x over previous
"""CrossAttentionBlock Trainium2 kernel (v2).

Math (reference):
    q = Wq@xq + bq        [RC=16, N]     (per-voxel 1x1x1 conv == channel matmul)
    k = Wk@xkv + bk       [16, N]
    v = Wv@xkv + bv       [C=128, N]
    S = (q^T k) / 4       [N, N]
    P = softmax_rows(S)
    out = v @ P^T         [C, N]
    y = x_q + gamma*out

Kernel strategy (8 NeuronCores, sequence-parallel over the N=13824 query
tokens; each core owns NQ=1728 queries against full K/V):
  * The hard throughput floor is PSUM->SBUF evacuation bandwidth: only the
    Activation and DVE engines can read PSUM, and every exp'd score element
    must cross once (exp is fused into the evacuation op, so exp itself is
    free).  Everything else is arranged to keep that path minimal:
      - v is never materialized: out = (gamma*Wv) @ (xkv @ P^T) reassociated,
        so the big [C,N] v evacuation disappears; Z = xkv @ exp(S^T)
        accumulates in PSUM via the same per-pair matmuls and is evacuated
        once per chunk ([C,432] instead of [C,N]).
      - the k projection packs 3 column-groups of 16 output rows into one
        [128,512] PSUM tile (base partitions 0/32/64), so its evacuation
        runs with full 128-lane utilization; a few SBUF->SBUF DMAs remap to
        the DoubleRow layout afterwards.
  * Scores are built TRANSPOSED (S^T tiles [128 keys x 432 queries]), fp8 +
    DoubleRow everywhere (2 MACs/cell/cycle): k is host-scaled by 16 into
    fp8 weights, so exp applies a 1/16 input scale (ScalarE scale arg /
    folded into the Schraudolph constant on DVE).  No max subtraction
    (|S|<~2 by construction); normalization deferred: Z and a ones-row
    matmul (ones=0.25 folds the Z evac scale) accumulate per chunk, then
    reciprocal + 1->128 broadcast matmul + multiply + residual add.
  * exp alternates ScalarE (true exp, fp8 out) / VectorE (Schraudolph int8
    bit-trick) ~53/47 Bresenham-interleaved.  Inputs land fp8/bf16 (xkv in
    both [c,m] and [m,c] layouts), chunked DMAs so projections overlap the
    loads.  Attention contributes O(1e-4) of the output, so fp8 noise is
    invisible; the residual is bf16 (0.2% of tolerance).
"""

import contextlib

import numpy as np
import ml_dtypes

import concourse.bass as bass
import concourse.mybir as mybir
from concourse import bacc
from concourse.tile import TileContext
from concourse.bass_utils import run_bass_kernel_spmd

F32 = mybir.dt.float32
BF16 = mybir.dt.bfloat16
FP8 = mybir.dt.float8e4
I8 = mybir.dt.int8
AF = mybir.ActivationFunctionType
DR = mybir.MatmulPerfMode.DoubleRow
ALU = mybir.AluOpType

C = 128           # channels
RC = 16           # reduced (q/k) channels
D = H = W = 24
N = D * H * W     # 13824 tokens
NCORES = 8
NQ = N // NCORES  # 1728 queries per core
CHUNK = 432       # query chunk ([128, CHUNK] f32 fits half a PSUM slot)
NCHUNKS = NQ // CHUNK   # 4
MT = N // 128     # 108 key tiles of 128
PAIRS = MT // 2   # 54 key-tile pairs per chunk
LAGP = 6          # Z/rs matmuls trail exp by this many pairs

KTW = 512         # k-projection column width per matmul (one PSUM bank)
KGROUPS = 3       # k output-row groups per PSUM tile (base partitions 0/32/64)
KTILES = N // (KTW * KGROUPS)   # 9

LOG2E = 1.4426950408889634
EXP8_SCALE = 8.0 * LOG2E / 16.0   # e4m3 bit trick, folding the S'=16*S scale
EXP8_BIAS = 56.0 - 0.3            # 7*8 + Schraudolph offset
ACT_FRAC = 0.531                  # ScalarE share of exp ops (1025/(905+1025))


def _act_pattern(n):
    pat, acc = [], 0.0
    for _ in range(n):
        acc += ACT_FRAC
        if acc >= 1.0:
            acc -= 1.0
            pat.append(True)
        else:
            pat.append(False)
    return pat

_BUILD_CACHE: dict = {}


def build_nc(repeats: int = 1):
    """Build + compile the per-core Bass program (SPMD across 8 cores)."""
    key = repeats
    if key in _BUILD_CACHE:
        return _BUILD_CACHE[key]

    nc = bacc.Bacc("TRN2", target_bir_lowering=False, debug=False,
                   num_devices=NCORES)
    wbf = nc.dram_tensor("wbf", [C, 2 * RC + C], BF16, kind="ExternalInput").ap()
    bias2 = nc.dram_tensor("bias2", [C, 3], F32, kind="ExternalInput").ap()
    xkv_f8 = nc.dram_tensor("xkv_f8", [C, N], FP8, kind="ExternalInput").ap()
    xq_bf = nc.dram_tensor("xq_bf", [C, NQ], BF16, kind="ExternalInput").ap()
    xkvT = nc.dram_tensor("xkvT", [C, N], FP8, kind="ExternalInput").ap()
    y = nc.dram_tensor("y", [C, NQ], BF16, kind="ExternalOutput").ap()

    with TileContext(nc) as tc, contextlib.ExitStack() as ctx:
        cpool = ctx.enter_context(tc.tile_pool(name="consts", bufs=1))
        ppool = ctx.enter_context(tc.tile_pool(name="psum", bufs=1, space="PSUM"))
        spool = ctx.enter_context(tc.tile_pool(name="work", bufs=1))

        # ---- resident inputs (issue order == HWDGE order: critical first) --
        KT1 = KTW * KGROUPS          # one k-tile's worth of xkv columns
        wbf_sb = cpool.tile([C, 2 * RC + C], BF16)
        nc.sync.dma_start(wbf_sb[:], wbf[:])
        bias_sb = cpool.tile([C, 3], F32)
        nc.sync.dma_start(bias_sb[:], bias2[:])
        xkv_sb = cpool.tile([C, N], FP8)
        # small chunks first so the k projection starts early
        nc.sync.dma_start(xkv_sb[:, 0:KT1], xkv_f8[:, 0:KT1])
        nc.sync.dma_start(xkv_sb[:, KT1:2 * KT1], xkv_f8[:, KT1:2 * KT1])
        xq_sb = cpool.tile([C, NQ], BF16)
        nc.sync.dma_start(xq_sb[:], xq_bf[:])
        XH = (N - 2 * KT1) // 2
        nc.sync.dma_start(xkv_sb[:, 2 * KT1:2 * KT1 + XH],
                          xkv_f8[:, 2 * KT1:2 * KT1 + XH])
        nc.sync.dma_start(xkv_sb[:, 2 * KT1 + XH:N], xkv_f8[:, 2 * KT1 + XH:N])
        xkvT_sb = cpool.tile([C, N], FP8)

        wqT = wbf_sb[:, 0:RC]
        wkT = wbf_sb[:, RC:2 * RC]
        wvT = wbf_sb[:, 2 * RC:2 * RC + C]
        bk16 = bias_sb[:, 0:1]
        bq_lo = bias_sb[0:8, 1:2]   # bq[p]/4 on partition p
        bq_hi = bias_sb[0:8, 2:3]   # bq[8+p]/4 on partition p

        ones_db = cpool.tile([C, 32], FP8)
        nc.gpsimd.memset(ones_db[:], 0.0625)   # folds the Z-evac 1/16 scale
        ones_row = cpool.tile([1, C], BF16)  # lhsT for 1->128 broadcast matmul
        nc.gpsimd.memset(ones_row[:], 1.0)
        warm_mv = cpool.tile([1, 512], BF16)
        nc.gpsimd.memset(warm_mv[:], 0.0)

        # PE p-state warmup: keep PE continuously busy from t~0 so the
        # projection matmuls run at full clock (ramp needs 3us of busy).
        warm_ps = ppool.tile([C, 512], F32, tag="rs", bufs=1, name="warm_ps")
        for _ in range(6):
            nc.tensor.matmul(warm_ps[0:1, :], ones_row[:, 0:1], warm_mv[:],
                             start=True, stop=True)

        # ---- projections ---------------------------------------------------
        # k': [128, KTILES*KTW] fp8, partition 32g+r holds 16*k[r] for column
        # group g; evacuations run full-width, then SBUF->SBUF DMAs remap to
        # the DoubleRow layout (o=0 on HWDGE/SP, o=1 on SWDGE/Pool).  q': two
        # 8-row matmuls per chunk-group write the DoubleRow halves side by
        # side in PSUM; 8-lane evacs land straight in q_db layout (no remap).
        # Only k tiles 0-1 and q group 0 run before the attention loop; the
        # rest is interleaved into chunk 0's pair pipeline below.
        k_sb = cpool.tile([C, KTILES * KTW], FP8)
        q_db = cpool.tile([8, 2 * NQ], FP8)
        qdv = q_db.rearrange("p (o g m) -> p o g m", o=2, g=NCHUNKS)
        k_db = cpool.tile([8, 2 * N], FP8)
        kv = k_sb.rearrange("p (t m) -> p t m", t=KTILES)
        kdv = k_db.rearrange("p (o t g m) -> p o t g m", o=2, t=KTILES, g=KGROUPS)

        def q_proj(g):
            psq = ppool.tile([C, 2 * KTW], F32, tag="st", bufs=3, name="psq")
            for o in range(2):
                nc.tensor.matmul(psq[0:8, 512 * o:512 * o + CHUNK],
                                 wqT[:, 8 * o:8 * o + 8],
                                 xq_sb[:, bass.ts(g, CHUNK)],
                                 start=True, stop=True)
            for o, b in ((0, bq_lo), (1, bq_hi)):
                src = psq[0:8, 512 * o:512 * o + CHUNK]
                if (2 * g + o) % 2 == 0:
                    nc.scalar.activation(qdv[:, o, g, :], src, AF.Identity,
                                         bias=b)
                else:
                    nc.vector.tensor_scalar(out=qdv[:, o, g, :], in0=src,
                                            scalar1=b, scalar2=None,
                                            op0=ALU.add)

        def k_tile(t):
            psk = ppool.tile([C, 2 * KTW], F32, tag="st", bufs=3, name="pk")
            for g in range(KGROUPS):
                lo = (KGROUPS * t + g) * KTW
                nc.tensor.matmul(psk[32 * g:32 * g + RC, 0:KTW],
                                 wkT, xkv_sb[:, lo:lo + KTW],
                                 start=True, stop=True)
            dst = k_sb[:, bass.ts(t, KTW)]
            if t % 2 == 0:
                nc.scalar.activation(dst, psk[:, 0:KTW], AF.Identity, bias=bk16)
            else:
                nc.vector.tensor_scalar(out=dst, in0=psk[:, 0:KTW],
                                        scalar1=bk16, scalar2=None, op0=ALU.add)

        def k_remap(t0, t1):
            tsl = slice(t0, t1)
            for g in range(KGROUPS):
                nc.sync.dma_start(kdv[:, 0, tsl, g, :],
                                  kv[32 * g:32 * g + 8, tsl, :])
                nc.gpsimd.dma_start(kdv[:, 1, tsl, g, :],
                                    kv[32 * g + 8:32 * g + 16, tsl, :])

        def xkvT_load(qq):
            sl = bass.ts(qq, N // 4)
            nc.sync.dma_start(xkvT_sb[:, sl], xkvT[:, sl])

        k_tile(0)
        k_tile(1)
        q_proj(0)
        k_remap(0, 2)
        xkvT_load(0)

        q3 = q_db.rearrange("p (o x) -> p o x", o=2)
        k3 = k_db.rearrange("p (o x) -> p o x", o=2)
        # remaining projection work, interleaved at chunk-0 pair slots
        extras = {
            0: [lambda: k_tile(2)], 2: [lambda: k_tile(3)],
            4: [lambda: k_remap(2, 4)], 5: [lambda: xkvT_load(1)],
            6: [lambda: k_tile(4)], 8: [lambda: k_tile(5)],
            10: [lambda: k_tile(6)], 12: [lambda: k_tile(7)],
            14: [lambda: k_tile(8)],
            15: [lambda: q_proj(1), lambda: k_remap(4, KTILES)],
            16: [lambda: xkvT_load(2)],
            17: [lambda: q_proj(2)],
            18: [lambda: xkvT_load(3)],
            19: [lambda: q_proj(3)],
        }

        # ---- attention main loop ------------------------------------------
        # Per-chunk epilogue is deferred into the NEXT chunk's pipeline so it
        # never head-of-line blocks the steady-state stream.
        act_pat = _act_pattern(NCHUNKS * PAIRS * max(repeats, 1))
        pend = {}

        def epi_a():
            # evacuate Z (frees its bank for the next chunk) + reciprocal
            pend["z8"] = z8 = spool.tile([C, CHUNK], FP8, tag="z8", bufs=2,
                                         name="z8")
            nc.scalar.activation(z8[:], pend.pop("z")[:], AF.Copy,
                                 scale=1.0 / 16.0)
            recip = spool.tile([1, CHUNK], F32, tag="recip", bufs=2)
            nc.vector.reciprocal_approx_fast(out=recip[:], in_=pend.pop("rs")[:])
            pend["recip_bf"] = recip_bf = spool.tile([1, CHUNK], BF16,
                                                     tag="recipb", bufs=2,
                                                     name="recip_bf")
            nc.gpsimd.tensor_copy(recip_bf[:], recip[:])

        def epi_m():
            # borrow one st slot: outu (cols 0:CHUNK) + bcp (cols 512:512+CHUNK)
            pend["pe_t"] = pet = ppool.tile([C, 1024], F32, tag="st", bufs=3,
                                            name="pe_t")
            nc.tensor.matmul(pet[:, 0:CHUNK], wvT, pend.pop("z8")[:],
                             start=True, stop=True)
            nc.tensor.matmul(pet[:, 512:512 + CHUNK], ones_row[:],
                             pend.pop("recip_bf")[:], start=True, stop=True)
            pend["outu_s"] = outu_s = spool.tile([C, CHUNK], F32, tag="outu_s",
                                                 bufs=2, name="outu_s")
            nc.scalar.copy(outu_s[:], pet[:, 0:CHUNK])

        def epi_b():
            sl = pend.pop("sl")
            pet = pend.pop("pe_t")
            t1 = spool.tile([C, CHUNK], BF16, tag="t1", bufs=2)
            nc.vector.tensor_tensor(out=t1[:], in0=pet[:, 512:512 + CHUNK],
                                    in1=pend.pop("outu_s")[:], op=ALU.mult)
            res = spool.tile([C, CHUNK], BF16, tag="res", bufs=2)
            nc.gpsimd.tensor_add(res[:], t1[:], xq_sb[:, sl])
            nc.sync.dma_start(y[:, sl], res[:])

        def epi_final(ch):
            # tail-latency version: two column halves pipelined across engines
            z, rs = pend.pop("z"), pend.pop("rs")
            HW_ = CHUNK // 2
            for h in range(2):
                lo = h * HW_
                z8h = spool.tile([C, HW_], FP8, tag="z8f", bufs=2, name="z8f")
                nc.scalar.activation(z8h[:], z[:, lo:lo + HW_], AF.Copy,
                                     scale=1.0 / 16.0)
                rch = spool.tile([1, HW_], F32, tag="recf", bufs=2, name="rcf")
                nc.vector.reciprocal_approx_fast(out=rch[:],
                                                 in_=rs[:, lo:lo + HW_])
                rbh = spool.tile([1, HW_], BF16, tag="rbf", bufs=2, name="rbf")
                nc.gpsimd.tensor_copy(rbh[:], rch[:])
                pet = ppool.tile([C, 1024], F32, tag="st", bufs=3, name="pe_f")
                nc.tensor.matmul(pet[:, 0:HW_], wvT, z8h[:],
                                 start=True, stop=True)
                nc.tensor.matmul(pet[:, 512:512 + HW_], ones_row[:], rbh[:],
                                 start=True, stop=True)
                osh = spool.tile([C, HW_], F32, tag="osf", bufs=2, name="osf")
                nc.scalar.copy(osh[:], pet[:, 0:HW_])
                t1h = spool.tile([C, HW_], BF16, tag="t1f", bufs=2, name="t1f")
                nc.vector.tensor_tensor(out=t1h[:], in0=pet[:, 512:512 + HW_],
                                        in1=osh[:], op=ALU.mult)
                rsh = spool.tile([C, HW_], BF16, tag="resf", bufs=2, name="rsf")
                cl = bass.ds(ch * CHUNK + lo, HW_)
                nc.gpsimd.tensor_add(rsh[:], t1h[:], xq_sb[:, cl])
                nc.sync.dma_start(y[:, cl], rsh[:])

        TOT = NCHUNKS * PAIRS
        o3 = ones_db.rearrange("p (b c) -> p b c", b=2)[:, :, 0:1]
        for rep in range(repeats):
            ex_tiles = {}
            zcur = {}
            for gp in range(TOT + LAGP):
                if rep == 0:
                    for f in extras.get(gp, ()):
                        f()
                s2 = gp - LAGP
                if s2 >= 0:
                    sp = s2 % PAIRS
                    if sp == 0 and "z" in pend:
                        epi_a()
                    if sp == 2 and "z8" in pend:
                        epi_m()
                    if sp == 4 and "pe_t" in pend:
                        epi_b()
                if gp < TOT:
                    ch = gp // PAIRS
                    s = gp % PAIRS
                    sl = bass.ts(ch, CHUNK)
                    stp = ppool.tile([C, 1024], F32, tag="st", bufs=3)
                    for j in range(2):
                        t = 2 * s + j
                        nc.tensor.matmul(stp[:, 512 * j:512 * j + CHUNK],
                                         k3[:, :, bass.ts(t, 128)],
                                         q3[:, :, sl],
                                         start=True, stop=True, perf_mode=DR)
                    st3 = stp.rearrange("p (b x) -> p b x", b=2)[:, :, 0:CHUNK]
                    ex = spool.tile([C, 2 * CHUNK], FP8, tag="ex", bufs=LAGP + 3)
                    ex3 = ex.rearrange("p (b x) -> p b x", b=2)
                    if act_pat[rep * TOT + gp]:
                        nc.scalar.activation(ex3, st3, AF.Exp, scale=1.0 / 16.0)
                    else:
                        nc.vector.tensor_scalar(
                            out=ex3.bitcast(I8), in0=st3,
                            scalar1=EXP8_SCALE, scalar2=EXP8_BIAS,
                            op0=ALU.mult, op1=ALU.add)
                    ex_tiles[gp] = ex
                if s2 >= 0:
                    ch2 = s2 // PAIRS
                    s = s2 % PAIRS
                    if s == 0:
                        zcur["z"] = ppool.tile([C, CHUNK], F32, tag="z",
                                               name="zpsum")
                        zcur["rs"] = ppool.tile([1, CHUNK], F32, tag="rs",
                                                name="rspsum")
                    ex = ex_tiles.pop(s2)
                    ex3 = ex.rearrange("p (b x) -> p b x", b=2)
                    xt3 = xkvT_sb[:, bass.ds(256 * s, 256)].rearrange(
                        "p (b c) -> p b c", b=2)
                    nc.tensor.matmul(zcur["z"][:], xt3, ex3, perf_mode=DR,
                                     start=(s == 0), stop=(s == PAIRS - 1))
                    nc.tensor.matmul(zcur["rs"][:], o3, ex3, perf_mode=DR,
                                     start=(s == 0), stop=(s == PAIRS - 1))
                    if s == PAIRS - 1:
                        pend.update(z=zcur.pop("z"), rs=zcur.pop("rs"),
                                    sl=bass.ts(ch2, CHUNK))
            if rep != repeats - 1:
                epi_a()
                epi_m()
                epi_b()
                tc.strict_bb_all_engine_barrier()
        if "z" in pend:
            pend.pop("sl")
            epi_final(NCHUNKS - 1)

    nc.compile()
    _BUILD_CACHE[key] = nc
    return nc


def _prep_in_maps(x_q, x_kv, Wq, bq, Wk, bk, Wv, bv, gamma):
    bf16 = ml_dtypes.bfloat16
    f8 = ml_dtypes.float8_e4m3
    f32 = np.float32
    x_q = np.asarray(x_q, f32).reshape(C, N)
    x_kv = np.asarray(x_kv, f32).reshape(C, N)
    Wq = np.asarray(Wq, f32)
    bq = np.asarray(bq, f32)
    Wk = np.asarray(Wk, f32)
    bk = np.asarray(bk, f32)
    Wv = np.asarray(Wv, f32)
    bv = np.asarray(bv, f32)
    gamma = float(np.asarray(gamma, f32).reshape(()))

    xkv_f8 = np.ascontiguousarray(x_kv).astype(f8)
    # xkv transposed [m, c] tiled by 128 keys (Z matmul stationary)
    xkvT = np.ascontiguousarray(
        x_kv.reshape(C, MT, 128).transpose(2, 1, 0).reshape(128, MT * C)).astype(f8)
    # bf16 weights blob: Wq^T/4 | 16*Wk^T | gamma*Wv^T
    wbf = np.zeros((C, 2 * RC + C), f32)
    wbf[:, 0:RC] = Wq.T * 0.25
    wbf[:, RC:2 * RC] = Wk.T * 16.0
    wbf[:, 2 * RC:2 * RC + C] = (gamma * Wv).T
    wbf = np.ascontiguousarray(wbf).astype(bf16)
    # biases: col0 = 16*bk tiled at rows 32g+r ; col1/col2 = bq/4 halves
    bias2 = np.zeros((C, 3), f32)
    for g in range(4):
        bias2[32 * g:32 * g + RC, 0] = 16.0 * bk
    bias2[0:8, 1] = 0.25 * bq[0:8]
    bias2[0:8, 2] = 0.25 * bq[8:16]
    resid_bias = (gamma * bv).astype(f32)  # softmax rows sum to 1

    in_maps = []
    for c in range(NCORES):
        xq_slice = np.ascontiguousarray(
            x_q[:, c * NQ:(c + 1) * NQ] + resid_bias[:, None]).astype(bf16)
        in_maps.append({
            "wbf": wbf, "bias2": bias2,
            "xkv_f8": xkv_f8, "xq_bf": xq_slice, "xkvT": xkvT,
        })
    return in_maps


def kernel(x_q, x_kv, Wq, bq, Wk, bk, Wv, bv, gamma):
    nc = build_nc(repeats=1)
    in_maps = _prep_in_maps(x_q, x_kv, Wq, bq, Wk, bk, Wv, bv, gamma)
    res = run_bass_kernel_spmd(nc, in_maps, list(range(NCORES)))
    out = np.concatenate([res.results[c]["y"].astype(np.float32)
                          for c in range(NCORES)], axis=1)
    return out.reshape(1, C, D, H, W).astype(np.float32)
